# revision 2
# baseline (speedup 1.0000x reference)
"""Trainium2 Bass kernel for nn_MetaMultiHeadSelfAttention_45810121179385 (v2).

Multi-head causal self-attention: B=4, S=2048, D=1024, H=16 heads (hd=64).

Sharding (8 NeuronCores): batch (4) x head-group (2 groups of 8 heads).
Core c handles batch b = c//2, head group g = c%2.

v2 structure (vs v1): single flat pipeline instead of 3 phases.
 - x resident in SBUF [128, 8, 2048] f32r; per-tile wq/wk slices loaded JIT.
 - q/k/v stored bf16 (scores/PV matmuls in bf16, 1 cyc/col, no f32r
   <256-col penalty); attention out + wo in bf16; projections stay f32r.
 - Scores computed per 1024-col psum piece, exp'd to bf16 e-tiles.
 - Normalization: DVE reciprocal of the ones-column row, gpsimd
   partition_broadcast across partitions (replaces v1's DRAM DMA bounce).
 - Projection groups for tile t+1, V groups, and o_proj groups are woven
   into the attention kt-loops as fillers so PE never idles waiting on
   the scores->exp->PV chain.
"""

import collections
import functools
import os
import sys

import numpy as np

sys.path.insert(0, "/opt/trn_rl_repo")

import concourse.bass as bass  # noqa: E402
import concourse.tile as tile  # noqa: E402
from concourse import bacc, mybir  # noqa: E402
from concourse.bass_utils import run_bass_kernel_spmd  # noqa: E402

F32 = mybir.dt.float32
F32R = mybir.dt.float32r
BF16 = mybir.dt.bfloat16
NPBF16 = mybir.dt.np(BF16)
EXP = mybir.ActivationFunctionType.Exp

B, S, D, H, HD = 4, 2048, 1024, 16, 64
NCORES = 8
HPC = 8          # heads per core
GD = HPC * HD    # 512 head-dims per core
NKT = S // 128   # 16 kpos tiles
NQC = S // 512   # 4 q chunks of 512
NDC = D // 128   # 8 contraction chunks for projections
NT = 4           # head-pair tiles per core
SCALE = 1.0 / np.sqrt(HD)

WEAVE = True
OPTS = {"fill_spread": 0, "oproj_split": 1, "carry_qc3": 1, "mask_pool": 1, "fill_at": (0, 2, 4, 8)}
VARIANTS = {
    "": {},
    "noweave": {},
    "fs0": {"fill_spread": 0},
    "os0": {"oproj_split": 0},
    "fs0os0": {"fill_spread": 0, "oproj_split": 0},
    "nocarry": {"carry_qc3": 0},
    "nomp": {"mask_pool": 0},
    "fa1357": {"fill_at": (1, 3, 5, 7)},
    "fa2581": {"fill_at": (2, 5, 8, 11)},
    "fa0246": {"fill_at": (0, 2, 4, 6)},
    "fa0248": {"fill_at": (0, 2, 4, 8)},
    "fa0369": {"fill_at": (0, 3, 6, 9)},
    "fa0124": {"fill_at": (0, 1, 2, 4)},
    "fa01210": {"fill_at": (0, 1, 2, 10)},
}


class Ctx:
    """Shared emission state."""

    def __init__(self, tc, pools, aps):
        self.tc = tc
        self.nc = tc.nc
        self.p = pools
        self.a = aps
        self.fillers = collections.deque()
        self.carry = None

    def fill(self, n=1):
        for _ in range(n):
            if not self.fillers:
                return
            self.fillers.popleft()()


def proj_group(cx, which, t, sc):
    """One [128,512] psum projection group: q or k for tile t, seq chunk sc."""
    nc = cx.nc
    x_sb = cx.a["x"]
    w_sb = cx.a["wq" if which == "q" else "wk"]
    ps = cx.p["ps"].tile([128, 1024], F32, tag="ps", name=f"ps_{which}{t}_{sc}")
    for k in range(NDC):
        nc.tensor.matmul(
            ps[:, 0:512],
            lhsT=w_sb[:, k, 128 * t : 128 * (t + 1)],
            rhs=x_sb[:, k, 512 * sc : 512 * (sc + 1)],
            start=(k == 0),
            stop=(k == NDC - 1),
        )
    dst = cx.a["qt" if which == "q" else "kt"]
    nc.vector.tensor_copy(
        out=dst[:, t, 512 * sc : 512 * (sc + 1)], in_=ps[:, 0:512]
    )


def v_group(cx, kti):
    """V projection for kpos tile kti (all 8 heads), scattered into v_sb."""
    nc = cx.nc
    x_sb, wv_sb, v_sb = cx.a["x"], cx.a["wv"], cx.a["v"]
    ps = cx.p["ps"].tile([128, 1024], F32, tag="ps", name=f"ps_v{kti}")
    for k in range(NDC):
        nc.tensor.matmul(
            ps[:, 0:512],
            lhsT=x_sb[:, k, 128 * kti : 128 * (kti + 1)],
            rhs=wv_sb[:, k, :],
            start=(k == 0),
            stop=(k == NDC - 1),
        )
    # all heads: data cols 0..63, ones col at 64
    nc.vector.tensor_copy(
        out=v_sb[:, kti, :, 0:HD],
        in_=ps[:, 0:512].rearrange("p (h d) -> p h d", h=HPC),
    )


def oproj_group(cx, m, qc):
    """o_proj partial for output tile m, query chunk qc."""
    nc = cx.nc
    wo_sb, ot_sb, yT = cx.a["wo"], cx.a["ot"], cx.a["yT"]
    ps = cx.p["ps"].tile([128, 1024], F32, tag="ps", name=f"ps_y{m}_{qc}")
    for t in range(NT):
        nc.tensor.matmul(
            ps[:, 0:512],
            lhsT=wo_sb[:, t, 128 * m : 128 * (m + 1)],
            rhs=ot_sb[:, t, 512 * qc : 512 * (qc + 1)],
            start=(t == 0),
            stop=(t == NT - 1),
        )
    y_t = cx.p["y"].tile([128, 512], F32, tag="y", name=f"y{m}_{qc}")
    nc.vector.tensor_copy(out=y_t, in_=ps[:, 0:512])
    nc.sync.dma_start(
        out=yT[128 * m : 128 * (m + 1), 512 * qc : 512 * (qc + 1)], in_=y_t
    )


def attention_head(cx, h, v_jit=False, post_kt=None):
    nc = cx.nc
    t_h, p_h = h // 2, 64 * (h % 2)
    qt, kt_sb, v_sb, ot_sb, mask_sb = (
        cx.a["qt"], cx.a["kt"], cx.a["v"], cx.a["ot"], cx.a["mask"],
    )
    e_tiles = {}

    def scores(kt):
        # exact-width e tile, alive until the qc3 PV burst reads it
        c_lo = 128 * kt
        e_t = cx.p["e"].tile(
            [128, S - c_lo], BF16, tag=f"e{kt}", name=f"e{h}_{kt}",
            bufs=2 if kt <= 2 else 1,
        )
        e_tiles[kt] = e_t
        a = c_lo
        while a < S:
            w = min(1024 - (a % 1024), S - a)
            ps = cx.p["ps"].tile([128, 1024], F32, tag="ps", name=f"ps_s{h}_{kt}")
            col = 0
            while col < w:
                n = min(512 - (col % 512), w - col)
                nc.tensor.matmul(
                    ps[:, col : col + n],
                    lhsT=kt_sb[p_h : p_h + 64, t_h, c_lo : c_lo + 128],
                    rhs=qt[p_h : p_h + 64, t_h, a + col : a + col + n],
                    start=True,
                    stop=True,
                )
                col += n
            nc.scalar.activation(
                out=e_t[:, a - c_lo : a - c_lo + w],
                in_=ps[:, 0:w],
                func=EXP,
                scale=SCALE,
            )
            a += w
        if OPTS["mask_pool"]:
            nc.gpsimd.tensor_mul(e_t[:, 0:128], e_t[:, 0:128], mask_sb)
        else:
            nc.vector.tensor_mul(e_t[:, 0:128], e_t[:, 0:128], mask_sb)

    def pv_burst(qc):
        # accumulate P@V for query chunk qc over kpos tiles 0..4*qc+3,
        # then normalize. pv psum lives only for this burst (pool bufs=2).
        q0 = 512 * qc
        pv = cx.p["pv"].tile([65, 512], F32, tag="pv", name=f"pv{h}_{qc}")
        last = 4 * qc + 3
        for kt in range(last + 1):
            c_lo = 128 * kt
            c0 = max(q0, c_lo)
            nc.tensor.matmul(
                pv[:, c0 - q0 : 512],
                lhsT=v_sb[:, kt, h, :],
                rhs=e_tiles[kt][:, c0 - c_lo : q0 + 512 - c_lo],
                start=(kt == 0),
                stop=(kt == last),
            )
        r_t = cx.p["r"].tile([65, 512], F32R, tag="r", name=f"r{h}_{qc}")
        with nc.allow_low_precision(reason="f32r is bit-identical to f32"):
            nc.vector.reciprocal(out=r_t[64:65, :], in_=pv[64:65, :])
        # broadcast the reciprocal row across partitions 0..63 via a
        # ones-column PE matmul (gpsimd partition_broadcast is sim-only)
        # broadcast the reciprocal row across partitions via a ones-column
        # PE matmul into a second pv-pool slot (matmul dst must be
        # partition-0 based), then bounce to SBUF (DVE reads one PSUM max)
        rb_ps = cx.p["pv"].tile([64, 512], F32, tag="pv", name=f"rbp{h}_{qc}")
        nc.tensor.matmul(
            rb_ps,
            lhsT=cx.a["ones"][64:65, :],
            rhs=r_t[64:65, :],
            start=True,
            stop=True,
        )
        rb_t = cx.p["r"].tile([64, 512], F32, tag="rb", name=f"rb{h}_{qc}")
        nc.vector.tensor_copy(out=rb_t, in_=rb_ps)
        if h % 2 == 0:
            nc.vector.tensor_mul(
                ot_sb[0:64, t_h, q0 : q0 + 512], pv[0:64, :], rb_t
            )
        else:
            st_t = cx.p["r"].tile([64, 512], BF16, tag="st", name=f"st{h}_{qc}")
            nc.vector.tensor_mul(st_t, pv[0:64, :], rb_t)
            nc.sync.dma_start(out=ot_sb[64:128, t_h, q0 : q0 + 512], in_=st_t)

    nf = len(cx.fillers)
    if OPTS["fill_at"] is not None and nf <= 4:
        fill_at = set(OPTS["fill_at"])
    elif OPTS["fill_spread"]:
        fill_at = (
            {round((i + 0.5) * NKT / nf) for i in range(nf)} if nf else set()
        )
    else:
        fill_at = set(range(NKT))
    for kt in range(NKT):
        scores(kt)
        if v_jit and kt + 2 < NKT:
            v_group(cx, kt + 2)  # V(0..3) were emitted upfront
        if kt == 1 and cx.carry is not None:
            cx.carry()
            cx.carry = None
        if kt in fill_at:
            cx.fill()
        if kt % 4 == 3 and (kt < 15 or not OPTS["carry_qc3"]):
            pv_burst(kt // 4)
        if post_kt is not None:
            post_kt(kt)
    if OPTS["carry_qc3"]:
        # defer the qc3 burst into the next head's window: it waits on this
        # head's last exp, and the next head's scores shouldn't queue behind it
        cx.carry = lambda: pv_burst(3)


def _mha_tile_kernel(tc, xT, wqT, wkT, wvT, woT, mask, onesT, yT):
    nc = tc.nc
    with (
        tc.tile_pool(name="resident", bufs=1) as rpool,
        tc.tile_pool(name="epool", bufs=1) as epool,
        tc.tile_pool(name="norm", bufs=2) as npool,
        tc.tile_pool(name="ypool", bufs=3) as ypool,
        tc.tile_pool(name="ps", bufs=3, space="PSUM") as pspool,
        tc.tile_pool(name="pspv", bufs=2, space="PSUM") as pvpool,
    ):
        x_sb = rpool.tile([128, NDC, S], BF16, tag="x")
        wq_sb = rpool.tile([128, NDC, GD], BF16, tag="wq")
        wk_sb = rpool.tile([128, NDC, GD], BF16, tag="wk")
        wv_sb = rpool.tile([128, NDC, GD], BF16, tag="wv")
        wo_sb = rpool.tile([128, NT, D], BF16, tag="wo")
        mask_sb = rpool.tile([128, 128], BF16, tag="mask")
        qt_sb = rpool.tile([128, NT, S], BF16, tag="qt")
        kt_sb = rpool.tile([128, NT, S], BF16, tag="kt")
        v_sb = rpool.tile([128, NKT, HPC, HD + 1], BF16, tag="v")
        ot_sb = rpool.tile([128, NT, S], BF16, tag="ot")

        pools = {"ps": pspool, "pv": pvpool, "e": epool, "r": npool, "y": ypool}
        aps = {
            "x": x_sb, "wq": wq_sb, "wk": wk_sb, "wv": wv_sb, "wo": wo_sb,
            "mask": mask_sb, "ones": None,
            "qt": qt_sb, "kt": kt_sb, "v": v_sb, "ot": ot_sb, "yT": yT,
        }
        cx = Ctx(tc, pools, aps)

        # ones column in v_sb (col 64, every head)
        nc.vector.tensor_copy(
            out=v_sb[:, :, :, HD : HD + 1],
            in_=nc.const_aps.tensor(1.0, [128, NKT, HPC, 1], F32),
        )
        # ones row at partition 64 (stationary for the reciprocal broadcast
        # matmul); loaded from DRAM since engines can't write cross-partition
        ones_sb = rpool.tile([65, 64], F32R, tag="ones")
        nc.sync.dma_start(out=ones_sb, in_=onesT)
        cx.a["ones"] = ones_sb

        # ---- input DMAs, priority order ----
        xT_r = xT.rearrange("(k p) s -> p k s", p=128)
        # x chunk 0 + wq interleaved in k-halves: the first proj group's
        # k-loop consumes chunks in order, so PE starts ~2.5us in.
        wqT_r = wqT.rearrange("(k p) g -> p k g", p=128)
        wkT_r = wkT.rearrange("(k p) g -> p k g", p=128)
        wvT_r = wvT.rearrange("(k p) g -> p k g", p=128)
        for lo, hi in ((0, 2), (2, 4), (4, 8)):
            nc.sync.dma_start(
                out=x_sb[:, lo:hi, 0:512], in_=xT_r[:, lo:hi, 0:512]
            )
            nc.sync.dma_start(out=wv_sb[:, lo:hi, :], in_=wvT_r[:, lo:hi, :])
        nc.sync.dma_start(out=wq_sb, in_=wqT_r)
        nc.sync.dma_start(out=wk_sb, in_=wkT_r)
        nc.sync.dma_start(out=x_sb[:, :, 512:1024], in_=xT_r[:, :, 512:1024])
        nc.sync.dma_start(out=x_sb[:, :, 1024:1536], in_=xT_r[:, :, 1024:1536])
        nc.sync.dma_start(out=x_sb[:, :, 1536:2048], in_=xT_r[:, :, 1536:2048])
        nc.sync.dma_start(out=mask_sb, in_=mask)
        nc.sync.dma_start(
            out=wo_sb, in_=woT.rearrange("(t p) m -> p t m", p=128)
        )

        # ---- upfront: tile-0 projections + first V groups ----
        # V(0..3) need only x chunk 0 + wv; interleave them between early
        # q/k groups so PE has work while later x chunks land.
        seq = [("v", 0), ("v", 1), ("v", 2), ("v", 3),
               ("q", 0), ("k", 0), ("q", 1), ("k", 1),
               ("q", 2), ("k", 2), ("q", 3), ("k", 3)]
        for which, i in seq:
            if which == "v":
                v_group(cx, i)
            else:
                proj_group(cx, which, 0, i)

        # ---- filler queues per head ----
        def make_proj_fillers(t):
            fs = []
            for sc in range(NQC):
                fs.append(lambda t=t, sc=sc: proj_group(cx, "q", t, sc))
                fs.append(lambda t=t, sc=sc: proj_group(cx, "k", t, sc))
            return fs

        f1 = make_proj_fillers(1)
        f2 = make_proj_fillers(2)
        f3 = make_proj_fillers(3)
        head_fillers = {
            1: f1,
            2: f2[:4],
            3: f2[4:],
            4: f3[:4],
            5: f3[4:],
        }

        def oproj_hook(kt):
            # qc's pv closes at kt=4qc+3; emit its o_proj 2 kt later so the
            # odd-head st->ot DMA has landed. qc3 is emitted after the loop.
            if OPTS["oproj_split"]:
                if kt in (5, 9, 13):
                    qc = (kt - 5) // 4
                    for m in range(4):
                        oproj_group(cx, m, qc)
                elif kt in (6, 10, 14):
                    qc = (kt - 6) // 4
                    for m in range(4, 8):
                        oproj_group(cx, m, qc)
            elif kt in (5, 9, 13):
                qc = (kt - 5) // 4
                for m in range(D // 128):
                    oproj_group(cx, m, qc)

        # odd (partition 64-127) head first in each pair: its st->ot DMA
        # overlaps the even head; the final head (6) writes ot via DVE
        # directly so the last o_proj chunk isn't gated on a DMA.
        head_order = [1, 0, 3, 2, 5, 4, 7, 6]
        for pos, h in enumerate(head_order):
            cx.fillers = collections.deque(head_fillers.get(pos, []))
            if not WEAVE:
                while cx.fillers:
                    cx.fillers.popleft()()
            attention_head(
                cx, h,
                v_jit=(pos == 0),
                post_kt=oproj_hook if pos == HPC - 1 else None,
            )
            # drain any leftover fillers at head end
            while cx.fillers:
                cx.fillers.popleft()()
        if cx.carry is not None:
            cx.carry()
            cx.carry = None
        for m in range(D // 128):
            oproj_group(cx, m, 3)


@functools.lru_cache(maxsize=8)
def build_program(variant=None):
    global WEAVE
    if variant is None:
        variant = os.environ.get("MHA_VARIANT", "")
    WEAVE = variant != "noweave"
    OPTS.clear()
    OPTS.update({"fill_spread": 0, "oproj_split": 1, "carry_qc3": 1,
                 "mask_pool": 1, "fill_at": (0, 2, 4, 8)})
    OPTS.update(VARIANTS.get(variant, {}))
    nc = bacc.Bacc("TRN2", target_bir_lowering=False, debug=False)
    xT = nc.dram_tensor("xT", [D, S], BF16, kind="ExternalInput").ap()
    wqT = nc.dram_tensor("wqT", [D, GD], BF16, kind="ExternalInput").ap()
    wkT = nc.dram_tensor("wkT", [D, GD], BF16, kind="ExternalInput").ap()
    wvT = nc.dram_tensor("wvT", [D, GD], BF16, kind="ExternalInput").ap()
    woT = nc.dram_tensor("woT", [GD, D], BF16, kind="ExternalInput").ap()
    mask = nc.dram_tensor("mask", [128, 128], BF16, kind="ExternalInput").ap()
    onesT = nc.dram_tensor("onesT", [65, 64], F32R, kind="ExternalInput").ap()
    yT = nc.dram_tensor("yT", [D, S], F32, kind="ExternalOutput").ap()
    with tile.TileContext(nc) as tc:
        _mha_tile_kernel(tc, xT, wqT, wkT, wvT, woT, mask, onesT, yT)
    nc.compile()
    return nc


def make_in_maps(x, q_proj, k_proj, v_proj, o_proj):
    x = np.ascontiguousarray(x, dtype=np.float32)
    mask = np.triu(np.ones((128, 128), dtype=np.float32))  # keep iff col >= row
    in_maps = []
    for c in range(NCORES):
        b, g = divmod(c, 2)
        sl = slice(GD * g, GD * (g + 1))
        in_maps.append(
            {
                "xT": np.ascontiguousarray(x[b].T).astype(NPBF16),
                "wqT": np.ascontiguousarray(
                    np.asarray(q_proj)[sl, :].T
                ).astype(NPBF16),
                "wkT": np.ascontiguousarray(
                    np.asarray(k_proj)[sl, :].T
                ).astype(NPBF16),
                "wvT": np.ascontiguousarray(
                    np.asarray(v_proj)[sl, :].T
                ).astype(NPBF16),
                "woT": np.ascontiguousarray(
                    np.asarray(o_proj)[:, sl].T
                ).astype(NPBF16),
                "mask": mask.astype(NPBF16),
                "onesT": np.concatenate(
                    [np.zeros((64, 64), np.float32), np.ones((1, 64), np.float32)]
                ),
            }
        )
    return in_maps


def gather_output(results):
    outs = [np.asarray(r["yT"], dtype=np.float32) for r in results]
    return np.stack(
        [(outs[2 * b] + outs[2 * b + 1]).T for b in range(B)], axis=0
    )


def kernel(x, q_proj, k_proj, v_proj, o_proj, _trace=False, _trace_kwargs=None):
    nc = build_program()
    in_maps = make_in_maps(x, q_proj, k_proj, v_proj, o_proj)
    res = run_bass_kernel_spmd(
        nc,
        in_maps,
        core_ids=list(range(NCORES)),
        trace=_trace,
        **(_trace_kwargs or {}),
    )
    y = gather_output(res.results)
    if _trace:
        kernel.last_result = res
    return y
